# revision 43
# baseline (speedup 1.0000x reference)
"""Distributed Trainium2 Bass kernel for the AstraGNN message-passing wrapper.

Math (per iteration, reference):
    m      = relu([h_src, h_dst] @ W1 + b1) @ W2 + b2        (per edge)
    m      = m * edge_mask
    agg    = segment_sum(m, dst)
    h      = relu([h, agg] @ Wu + bu)
    logits = h @ Wo + bo                                      (returned for last iter)

Kernel reformulation:
    [h_src, h_dst] @ W1 = (h @ W1a)[src] + (h @ W1b)[dst]  with W1 = [W1a; W1b]
    segment_sum(relu(...) @ W2) = segment_sum(relu(...)) @ W2   (W2 linear)
    => per-edge work reduces to: gather P[src], add Q[dst], relu, segment-sum.
    The segment-sum and the @W2 both happen on the TensorEngine via PSUM
    accumulation over fixed-size "slot blocks" of a degree-padded edge grid.

Distribution: nodes are sharded over 8 cores (dst-owner sharding).  Each core
computes P = h @ W1a for its shard in a node-major block layout; an AllGather
replicates the P blob to all cores' DRAM; each core then loads the blob into
SBUF (a straight, line-rate DMA thanks to the partition-major blob layout)
and gathers P[src] for its own edges with SBUF-SOURCE dma_gather (transpose
mode).  Sourcing the gather from SBUF avoids the ~430ns random-read HBM
latency that limits HBM-source gathers to ~2.4B/ns per DMA engine.

SBUF table layout (sbuf_tokens_per_rank=128, sbuf_free_dim_per_rank=256):
gather index i reads partition i%128, free bytes [(i//128)*256, +256).
Node with core c, local label l sits at partition l%128, rank c_rel*nblk +
l//128.  The int16 index limit (32767) is handled with TWO SBUF table tiles:
part A holds cores 0-4 (+ a -1e4 pad rank), part B cores 5-7 (+ pad rank);
the A/B split is by tile base address.

Masked edges are dropped on the host.  Edges are laid out host-side as a
per-dst-tile slot grid: tile = NDST_TILE destination nodes, D slot-blocks;
block c holds the c-th in-edge of every dst in the tile (missing -> the pad
rank, whose -1e4 relu kills).  Within each core, nodes are ordered by
(-degA//4, -degB), which packs similar-degree dsts into the same tile and
cuts slot padding ~18% vs a plain degA sort.
"""

import sys

sys.path.insert(0, "/opt/trn_rl_repo")

import numpy as np
import ml_dtypes

import concourse.bass as bass
import concourse.mybir as mybir
from concourse import bacc
from concourse.tile import TileContext
from concourse.bass_utils import run_bass_kernel_spmd

BF16 = ml_dtypes.bfloat16
NCORES = 8
H = 128
ITERS = 3
NDST_TILE = 256
CHUNK_IDX = 768  # idxs per call; nb*w must be %128 and fit the ring (~1024)
DMA_SCRATCH = 16384  # HW ring is fixed ~1024 descs/queue; keep default
NEG_BIG = -10000.0
A_CORES = 5


# ---------------------------------------------------------------- host side

def _preprocess(x_nodes, edge_index, edge_mask):
    N = x_nodes.shape[0]
    src = np.asarray(edge_index[0], dtype=np.int64)
    dst = np.asarray(edge_index[1], dtype=np.int64)
    em = np.asarray(edge_mask, dtype=bool)
    src, dst = src[em], dst[em]

    nloc = ((N + NCORES - 1) // NCORES + 127) // 128 * 128  # per-core padded
    nblk = nloc // 128
    # --- shard assignment: sort by in-degree, round-robin over cores
    indeg = np.bincount(dst, minlength=N)
    order = np.argsort(-indeg, kind="stable")
    rank = np.empty(N, dtype=np.int64)
    rank[order] = np.arange(N)
    core = rank % NCORES
    pos0 = rank // NCORES

    # --- A/B split by src core (A = cores 0-4, B = cores 5-7)
    assert A_CORES * nloc + 128 <= 32767
    assert (NCORES - A_CORES) * nloc + 128 <= 32767
    is_a_src = core[src] < A_CORES

    # per-dst degrees within each part (preliminary labels)
    glob0 = core * nloc + pos0
    degA = np.bincount(glob0[dst][is_a_src], minlength=NCORES * nloc)
    degB = np.bincount(glob0[dst][~is_a_src], minlength=NCORES * nloc)

    # --- within-core ordering: banded (-degA//4) primary, -degB secondary
    final_pos = np.empty(N, dtype=np.int64)
    for c in range(NCORES):
        nodes = np.nonzero(core == c)[0]
        ga, gb = degA[glob0[nodes]], degB[glob0[nodes]]
        perm = np.lexsort((-gb, -(ga // 4)))
        final_pos[nodes[perm]] = np.arange(nodes.size)
    label = core * nloc + final_pos  # final label of each original node
    src_l, dst_l = label[src], label[dst]

    # --- tiles (uniform across cores)
    ntiles_full = nloc // NDST_TILE
    tile_sizes = [NDST_TILE] * ntiles_full
    if nloc % NDST_TILE:
        tile_sizes.append(nloc % NDST_TILE)
    tile_starts = np.concatenate([[0], np.cumsum(tile_sizes)])[:-1]

    # per (core, tile) max degree in each part -> uniform max over cores
    dloc = dst_l % nloc
    dcore = dst_l // nloc
    degA_l = np.bincount(dst_l[is_a_src], minlength=NCORES * nloc).reshape(NCORES, nloc)
    degB_l = np.bincount(dst_l[~is_a_src], minlength=NCORES * nloc).reshape(NCORES, nloc)
    DA, DB = [], []
    for t0, w in zip(tile_starts, tile_sizes):
        DA.append(int(degA_l[:, t0 : t0 + w].max()))
        DB.append(int(degB_l[:, t0 : t0 + w].max()))

    # --- build the per-core slot grids and the flat index stream
    # call list (shared across cores): (tile, part, c0, nb, w, colbase)
    calls = []
    colbase = 0
    for ti, (t0, w) in enumerate(zip(tile_starts, tile_sizes)):
        for part, D in (("A", DA[ti]), ("B", DB[ti])):
            nb_max = max(1, CHUNK_IDX // w)
            c0 = 0
            while c0 < D:
                nb = min(nb_max, D - c0)
                calls.append((ti, part, c0, nb, w, colbase))
                colbase += nb * w // 16
                c0 += nb
    totc = colbase

    # interleave A/B calls per tile so both drain paths stay in flight
    by_tile = {}
    for cl in calls:
        by_tile.setdefault((cl[0], cl[1]), []).append(cl)
    merged = []
    for ti in range(len(tile_sizes)):
        a = by_tile.get((ti, "A"), [])
        b = by_tile.get((ti, "B"), [])
        for i in range(max(len(a), len(b))):
            if i < len(a):
                merged.append(a[i])
            if i < len(b):
                merged.append(b[i])
    calls = merged

    # gather index: node (c, l) -> rank c_rel*nblk + l//128, token l%128,
    # within its part's SBUF table (A = cores 0-4, B = cores 5-7).
    lq, lr = src_l % nloc // 128, src_l % nloc % 128
    c_of = src_l // nloc
    idx_a_val = ((c_of * nblk + lq) * 128 + lr)
    idx_b_val = ((c_of - A_CORES) * nblk + lq) * 128 + lr
    n_use = N // NCORES  # used labels per core; [n_use, nloc) hold NEG_BIG
    assert n_use * NCORES == N and n_use < nloc
    pad_a = A_CORES * nloc      # rank A_CORES*nblk, token 0 (NEG rank)
    pad_b = (NCORES - A_CORES) * nloc

    idx_streams = []
    cnt_arr = np.zeros((NCORES, nloc), np.float32)
    for c in range(NCORES):
        m = dcore == c
        dl, ia = dloc[m], is_a_src[m]
        va, vb = idx_a_val[m], idx_b_val[m]
        cnt_arr[c] = np.bincount(dl, minlength=nloc)
        stream = np.empty(totc * 16, np.int16)
        for part in ("A", "B"):
            sel = ia if part == "A" else ~ia
            d_p = dl[sel]
            s_p = (va if part == "A" else vb)[sel]
            o = np.lexsort((s_p, d_p))
            d_p, s_p = d_p[o], s_p[o]
            # slot within dst: running count
            first = np.concatenate([[True], d_p[1:] != d_p[:-1]])
            gstart = np.nonzero(first)[0]
            slot = np.arange(d_p.size) - np.repeat(gstart, np.diff(np.concatenate([gstart, [d_p.size]])))
            val = s_p.astype(np.int16)
            # grid[tile][slot, dst_in_tile]
            for ti, (t0, w) in enumerate(zip(tile_starts, tile_sizes)):
                D = DA[ti] if part == "A" else DB[ti]
                if D == 0:
                    continue
                grid = np.full((D, w), pad_a if part == "A" else pad_b, np.int16)
                mm = (d_p >= t0) & (d_p < t0 + w)
                grid[slot[mm], d_p[mm] - t0] = val[mm]
                for (ti2, part2, c0, nb, w2, cb) in calls:
                    if ti2 == ti and part2 == part:
                        stream[cb * 16 : cb * 16 + nb * w] = grid[c0 : c0 + nb].ravel()
        wrapped = stream.reshape(totc, 16).T  # [16, totc]
        idx_streams.append(np.tile(wrapped, (8, 1)))  # [128, totc]

    meta = dict(
        N=N, nloc=nloc, nblk=nblk, n_use=n_use,
        tile_sizes=tile_sizes, tile_starts=list(tile_starts),
        DA=DA, DB=DB, calls=calls, totc=totc,
    )
    return meta, idx_streams, cnt_arr, label


# ------------------------------------------------------------- device side

_PROGRAM_CACHE = {}


def _build_program(meta):
    key = (meta["N"], meta["totc"], tuple(meta["DA"]), tuple(meta["DB"]))
    if key in _PROGRAM_CACHE:
        return _PROGRAM_CACHE[key]

    nloc = meta["nloc"]
    nblk = meta["nblk"]
    n_use = meta["n_use"]
    tile_sizes = meta["tile_sizes"]
    tile_starts = meta["tile_starts"]
    calls = meta["calls"]
    totc = meta["totc"]
    bf = mybir.dt.bfloat16
    f32 = mybir.dt.float32
    AF = mybir.ActivationFunctionType
    ranks_a = A_CORES * nblk + 1          # + NEG pad rank
    ranks_b = (NCORES - A_CORES) * nblk + 1

    nc = bacc.Bacc("TRN2", target_bir_lowering=False, debug=False,
                   num_devices=NCORES, num_swdge_queues=4,
                   dynamic_dma_scratch_size=DMA_SCRATCH)

    # external I/O
    hT0_e = nc.dram_tensor("hT0", [128, nloc], bf, kind="ExternalInput")
    idx_e = nc.dram_tensor("idx", [128, totc], mybir.dt.int16, kind="ExternalInput")
    cnt_e = nc.dram_tensor("cnt", [1, nloc], bf, kind="ExternalInput")
    w1a_e = nc.dram_tensor("W1a", [128, 128], bf, kind="ExternalInput")
    w1b_e = nc.dram_tensor("W1b", [128, 128], bf, kind="ExternalInput")
    w2_e = nc.dram_tensor("W2", [128, 128], bf, kind="ExternalInput")
    wut_e = nc.dram_tensor("Wut", [128, 128], bf, kind="ExternalInput")
    wub_e = nc.dram_tensor("Wub", [128, 128], bf, kind="ExternalInput")
    wo_e = nc.dram_tensor("Wo", [128, 2], bf, kind="ExternalInput")
    b1_e = nc.dram_tensor("b1", [128, 1], f32, kind="ExternalInput")
    bu_e = nc.dram_tensor("bu", [1, 128], bf, kind="ExternalInput")
    b2_e = nc.dram_tensor("b2", [1, 128], bf, kind="ExternalInput")
    bo_e = nc.dram_tensor("bo", [1, 2], bf, kind="ExternalInput")
    out_e = nc.dram_tensor("out", [nloc, 2], f32, kind="ExternalOutput")

    # internal DRAM: partition-major P blob, AllGathered across cores
    ptable = nc.dram_tensor("ptable", [NCORES * 128, nblk * 128], bf,
                            addr_space="Shared")
    ag_in = nc.dram_tensor("ag_in", [128, nblk * 128], bf)

    with TileContext(nc) as tc:
        with (
            tc.tile_pool(name="res", bufs=1) as res,
            tc.tile_pool(name="gpool", bufs=8) as gpool,
            tc.tile_pool(name="spool", bufs=3) as spool,
            tc.tile_pool(name="cpool", bufs=3) as cpool,
            tc.tile_pool(name="pe_psum", bufs=3, space="PSUM") as pe_psum,
            tc.tile_pool(name="pa_psum", bufs=2, space="PSUM") as pa_psum,
            tc.tile_pool(name="pu_psum", bufs=2, space="PSUM") as pu_psum,
        ):
            # ---- residents
            idx_sb = res.tile([128, totc], mybir.dt.int16, tag="idx")
            hT = [res.tile([128, nloc], bf, tag=f"hT{i}", name=f"hT{i}")
                  for i in range(2)]
            tblA = res.tile([128, ranks_a * 128], bf, tag="tblA")
            tblB = res.tile([128, ranks_b * 128], bf, tag="tblB")
            Q_sb = res.tile([128, nloc], bf, tag="Q")
            P_nm = res.tile([128, nblk, 128], bf, tag="Pnm")
            w1a = res.tile([128, 128], bf, tag="w1a")
            w1b = res.tile([128, 128], bf, tag="w1b")
            w2 = res.tile([128, 128], bf, tag="w2")
            wut = res.tile([128, 128], bf, tag="wut")
            wub = res.tile([128, 128], bf, tag="wub")
            wo = res.tile([128, 2], bf, tag="wo")
            b1 = res.tile([128, 1], f32, tag="b1")
            bu = res.tile([1, 128], bf, tag="bu")
            b2 = res.tile([1, 128], bf, tag="b2")
            bo = res.tile([1, 2], bf, tag="bo")
            ones = res.tile([1, NDST_TILE], bf, tag="ones")
            lst = res.tile([128, nblk * 2], f32, tag="lst")

            for t, e in [(idx_sb, idx_e), (hT[0], hT0_e),
                         (w1a, w1a_e), (w1b, w1b_e), (w2, w2_e), (wut, wut_e),
                         (wub, wub_e), (wo, wo_e), (b1, b1_e), (bu, bu_e),
                         (b2, b2_e), (bo, bo_e)]:
                nc.sync.dma_start(out=t[:], in_=e.ap())
            nc.vector.memset(ones[:], 1.0)
            nc.vector.memset(tblA[:, (A_CORES * nblk) * 128 :], NEG_BIG)
            nc.vector.memset(tblB[:, ((NCORES - A_CORES) * nblk) * 128 :], NEG_BIG)

            qrot = [0]
            out_3d = out_e.ap().rearrange("(b p) o -> p b o", p=128)
            ptable_v = ptable.ap().rearrange("(c p) x -> p c x", p=128)

            for it in range(ITERS):
                h = hT[it % 2]
                hn = hT[(it + 1) % 2]

                # ---- phase A: node-major P blocks + feature-major Q
                for b in range(nblk):
                    blk = slice(b * 128, (b + 1) * 128)
                    ps = pa_psum.tile([128, 512], f32, tag="psA")
                    nc.tensor.matmul(out=ps[:, 0:128], lhsT=h[:, blk],
                                     rhs=w1a[:], start=True, stop=True)
                    nc.scalar.activation(out=P_nm[:, b, :], in_=ps[:, 0:128],
                                         func=AF.Copy)
                    # per-block blob store: lets the AllGather start right
                    # after the last block instead of after one big DMA
                    nc.sync.dma_start(
                        out=ag_in.ap()[:, b * 128 : (b + 1) * 128],
                        in_=P_nm[:, b, :])
                for q0 in range(0, nloc, 512):
                    qw = min(512, nloc - q0)
                    ps = pa_psum.tile([128, 512], f32, tag="psA")
                    nc.tensor.matmul(out=ps[:, :qw], lhsT=w1b[:],
                                     rhs=h[:, q0 : q0 + qw], start=True, stop=True)
                    nc.scalar.activation(out=Q_sb[:, q0 : q0 + qw], in_=ps[:, :qw],
                                         func=AF.Identity, bias=b1[:])
                if True:  # AllGather P and refresh the SBUF tables
                    nc.gpsimd.collective_compute(
                        "AllGather",
                        mybir.AluOpType.bypass,
                        replica_groups=[list(range(NCORES))],
                        ins=[ag_in.ap().opt()],
                        outs=[ptable.ap().opt()],
                    )
                    # load the A part into the SBUF table (straight big DMA)
                    nc.sync.dma_start(
                        out=tblA[:, : A_CORES * nblk * 128]
                            .rearrange("p (c x) -> p c x", c=A_CORES),
                        in_=ptable_v[:, 0:A_CORES, :],
                    )
                    nc.sync.dma_start(
                        out=tblB[:, : (NCORES - A_CORES) * nblk * 128]
                            .rearrange("p (c x) -> p c x", c=NCORES - A_CORES),
                        in_=ptable_v[:, A_CORES:NCORES, :],
                    )

                # ---- phase B: edge grid -> PSUM agg -> node update
                for ti, (t0, w) in enumerate(zip(tile_starts, tile_sizes)):
                    tile_calls = [cl for cl in calls if cl[0] == ti]
                    nq = w // 128
                    qb0 = t0 // 128
                    psE = pe_psum.tile([128, NDST_TILE], f32, tag="psE")
                    cnt_t = cpool.tile([1, NDST_TILE], bf, tag="cnt")
                    nc.sync.dma_start(out=cnt_t[:, :w],
                                      in_=cnt_e.ap()[:, t0 : t0 + w])
                    # seed with b2 * cnt (each real edge contributes b2)
                    nc.tensor.matmul(out=psE[:, :w], lhsT=b2[:],
                                     rhs=cnt_t[:, :w],
                                     start=True, stop=not tile_calls)
                    for ci, (_, part, c0, nb, _, cb) in enumerate(tile_calls):
                        nidx = nb * w
                        last_call = ci + 1 == len(tile_calls)
                        g = gpool.tile([128, CHUNK_IDX], bf, tag="g")
                        g3 = g[:, :nidx].rearrange("p (b d) -> p b d", d=w)
                        qb = Q_sb[:, t0 : t0 + w].unsqueeze(1).to_broadcast(
                            [128, nb, w])
                        nc.gpsimd.dma_gather(
                            g[:, :nidx].rearrange("p (a n) -> p a n", a=1),
                            tblA[:] if part == "A" else tblB[:],
                            idx_sb[:, cb : cb + nidx // 16],
                            num_idxs=nidx,
                            num_idxs_reg=nidx,
                            elem_size=128,
                            transpose=True,
                            queue_num=qrot[0],
                            sbuf_tokens_per_rank=128,
                            sbuf_free_dim_per_rank=256,
                            sbuf_free_dim_pad_per_rank=0,
                            sbuf_byte_offset=0,
                        )
                        qrot[0] = (qrot[0] + 1) % 4
                        nc.vector.tensor_tensor(out=g3, in0=g3, in1=qb,
                                                op=mybir.AluOpType.add)
                        nc.scalar.activation(out=g3, in_=g3, func=AF.Relu)
                        for b in range(nb):
                            nc.tensor.matmul(
                                out=psE[:, :w], lhsT=w2[:],
                                rhs=g3[:, b, :],
                                start=False,
                                stop=last_call and b == nb - 1,
                            )
                    # ---- node update for this tile
                    agg = spool.tile([128, NDST_TILE], bf, tag="agg")
                    nc.vector.tensor_copy(out=agg[:, :w], in_=psE[:, :w])
                    psU = pu_psum.tile([128, NDST_TILE], f32, tag="psU")
                    nc.tensor.matmul(out=psU[:, :w], lhsT=wut[:],
                                     rhs=h[:, t0 : t0 + w],
                                     start=True, stop=False)
                    nc.tensor.matmul(out=psU[:, :w], lhsT=wub[:],
                                     rhs=agg[:, :w], start=False, stop=False)
                    nc.tensor.matmul(out=psU[:, :w], lhsT=bu[:],
                                     rhs=ones[:, :w], start=False, stop=True)
                    nc.scalar.activation(out=hn[:, t0 : t0 + w], in_=psU[:, :w],
                                         func=AF.Relu)

            # ---- output head on final h
            hfin = hT[ITERS % 2]
            for b in range(nblk):
                ps = pa_psum.tile([128, 512], f32, tag="psA")
                nc.tensor.matmul(out=ps[:, 0:2],
                                 lhsT=hfin[:, b * 128 : (b + 1) * 128],
                                 rhs=wo[:], start=True, stop=False)
                nc.tensor.matmul(out=ps[:, 0:2], lhsT=ones[:, :128], rhs=bo[:],
                                 start=False, stop=True)
                nc.vector.tensor_copy(out=lst[:, b * 2 : b * 2 + 2],
                                      in_=ps[:, 0:2])
            nc.sync.dma_start(out=out_3d,
                              in_=lst[:].rearrange("p (b o) -> p b o", o=2))

    nc.compile()
    _PROGRAM_CACHE[key] = nc
    return nc


# --------------------------------------------------------------- interface

def kernel(x_nodes, edge_index, edge_attr, node_mask, edge_mask,
           W1, b1, W2, b2, Wu, bu, Wo, bo):
    x_nodes = np.asarray(x_nodes, dtype=np.float32)
    meta, idx_streams, cnt_arr, label = _preprocess(x_nodes, edge_index, edge_mask)
    nloc = meta["nloc"]
    N = meta["N"]

    nc = _build_program(meta)

    W1 = np.asarray(W1, np.float32)
    Wu = np.asarray(Wu, np.float32)
    shared = dict(
        W1a=W1[:H].astype(BF16), W1b=W1[H:].astype(BF16),
        W2=np.asarray(W2, np.float32).astype(BF16),
        Wut=Wu[:H].astype(BF16), Wub=Wu[H:].astype(BF16),
        Wo=np.asarray(Wo, np.float32).astype(BF16),
        b1=np.asarray(b1, np.float32).reshape(128, 1),
        bu=np.asarray(bu, np.float32).reshape(1, 128).astype(BF16),
        b2=np.asarray(b2, np.float32).reshape(1, 128).astype(BF16),
        bo=np.asarray(bo, np.float32).reshape(1, 2).astype(BF16),
    )

    in_maps = []
    for c in range(NCORES):
        hT0 = np.zeros((128, nloc), BF16)
        sel = (label // nloc) == c
        hT0[:, label[sel] % nloc] = x_nodes[sel].T.astype(BF16)
        in_maps.append(dict(
            hT0=hT0,
            idx=idx_streams[c],
            cnt=cnt_arr[c].reshape(1, nloc).astype(BF16),
            **shared,
        ))

    global _last_in_maps
    _last_in_maps = in_maps
    import os
    trace = bool(os.environ.get("KERNEL_TRACE"))
    res = run_bass_kernel_spmd(nc, in_maps, core_ids=list(range(NCORES)),
                               trace=trace)
    if trace:
        print(f"HW exec time: {res.exec_time_ns} ns")
    full = np.concatenate([r["out"] for r in res.results], axis=0)  # [8*nloc, 2]
    by_label = full.reshape(NCORES * nloc, 2)
    return by_label[label].astype(np.float32)
